# revision 56
# baseline (speedup 1.0000x reference)
"""Causal multi-head attention on 8 Trainium2 NeuronCores.

Sharding: core c -> (batch g = c // 4, head-group p = c % 4, heads 4p..4p+3).
Each core projects Q/K/V for its batch with its 256 feature columns
(column-sharded w_q/w_k/w_v), runs causal attention for its 4 heads, computes
the partial output projection with its 256 rows of w_o, and a ReduceScatter
over each batch group sums the partials and hands every core its own 512-row
output shard.

v2 design (everything bf16 into the PE, fp32 accumulation):
- Exact 128-granularity causality: scores strips [128 kpos, q>=128*ki] per
  (head, k-tile), 53% of the S^2 matrix instead of 62.5%.
- Flipped AV: ctx[q-part, 65] += pr_block.T @ v_aug where v_aug is V in
  natural [kpos, feat] layout with an appended ones column -> softmax
  denominator lands in psum column 64, per-PARTITION normalize (cheap).
- V projected directly in natural layout (no PE transpose pass).
- b_k dropped entirely (softmax-shift invariant), b_v folded into the output
  bias host-side (bv @ w_o), b_q applied as the exp() per-partition bias via
  k.bq matvecs.
- Streamed tail: per q-tile normalize -> PE transpose -> output projection
  -> rs_in DMA, overlapped under the attention exp stream.
"""

import numpy as np

B, S, D, H = 2, 2048, 1024, 16
DK = D // H  # 64
N_CORES = 8
FPC = 256  # features per core
NKT = S // 128  # 16 k/q tiles

_CACHE = {}


def _build_nc():
    import os as os_mod
    import concourse.mybir as mybir
    import concourse.tile as tile
    from concourse import bacc

    F32 = mybir.dt.float32
    BF16 = mybir.dt.bfloat16
    Exp = mybir.ActivationFunctionType.Exp

    nc = bacc.Bacc("TRN2", target_bir_lowering=False, debug=False, num_devices=8)

    xq = nc.dram_tensor("xq", [D, S], BF16, kind="ExternalInput")
    xk = nc.dram_tensor("xk", [D, S], BF16, kind="ExternalInput")
    xv = nc.dram_tensor("xv", [D, S], BF16, kind="ExternalInput")
    wq = nc.dram_tensor("wq", [D, FPC], BF16, kind="ExternalInput")
    wk = nc.dram_tensor("wk", [D, FPC], BF16, kind="ExternalInput")
    wv = nc.dram_tensor("wv", [D, FPC], BF16, kind="ExternalInput")
    wo = nc.dram_tensor("wo", [FPC, D], BF16, kind="ExternalInput")
    bq = nc.dram_tensor("bq", [FPC, 1], F32, kind="ExternalInput")
    bo1 = nc.dram_tensor("bo1", [1, D], BF16, kind="ExternalInput")
    bo4 = nc.dram_tensor("bo4", [128, D], F32, kind="ExternalInput")
    tri = nc.dram_tensor("tri", [128, 128], BF16, kind="ExternalInput")
    ident = nc.dram_tensor("ident", [128, 128], BF16, kind="ExternalInput")
    out = nc.dram_tensor("out", [512, D], F32, kind="ExternalOutput")

    dbg = bool(os_mod.environ.get("BASS_DEBUG_DUMP"))
    if dbg:
        dbg_qT = nc.dram_tensor("dbg_qT", [128, 2, S], BF16, kind="ExternalOutput")
        dbg_kT = nc.dram_tensor("dbg_kT", [128, 2, S], BF16, kind="ExternalOutput")
        dbg_v = nc.dram_tensor("dbg_v", [128, NKT, 4 * 65], BF16, kind="ExternalOutput")
        dbg_ctxn = nc.dram_tensor("dbg_ctxn", [128, 2, NKT, 128], BF16, kind="ExternalOutput")
        dbg_ctxT = nc.dram_tensor("dbg_ctxT", [128, 2, S], BF16, kind="ExternalOutput")

    from contextlib import ExitStack
    with tile.TileContext(nc) as tc:
        with (
            tc.tile_pool(name="consts", bufs=1) as consts,
            tc.tile_pool(name="persist", bufs=1) as persist,
            tc.tile_pool(name="xin", bufs=3) as xin,
            tc.tile_pool(name="probs", bufs=26) as probs,
            tc.tile_pool(name="small", bufs=6) as small,
            tc.tile_pool(name="oout", bufs=4) as oout,
            tc.tile_pool(name="dram", bufs=1, space="DRAM") as dram,
        ):
            # ---- constant tiles ----
            wq_s = consts.tile([128, 8, FPC], BF16, tag="wq")
            wk_s = consts.tile([128, 8, FPC], BF16, tag="wk")
            wv_s = consts.tile([128, 8, FPC], BF16, tag="wv")
            wo_s = consts.tile([128, 2, D], BF16, tag="wo")
            bq_s = consts.tile([128, 2], F32, tag="bq")
            bo1_s = consts.tile([1, D], BF16, tag="bo1")
            bo4_s = consts.tile([128, D], F32, tag="bo4")
            ones1_s = consts.tile([1, 128], BF16, tag="ones1")
            tri_s = consts.tile([128, 128], BF16, tag="tri")
            ident_s = consts.tile([128, 128], BF16, tag="ident")
            nc.vector.memset(ones1_s[:], 1.0)


            # ---- persistent activations ----
            # qT/kT: [head-dim on partitions (2 heads: rows 0:64 / 64:128), S]
            qT_s = [persist.tile([128, S], BF16, tag=f"qT{i}", name=f"qT{i}") for i in range(2)]
            kT_s = [persist.tile([128, S], BF16, tag=f"kT{i}", name=f"kT{i}") for i in range(2)]
            # V natural layout + ones column: per ki, head h strip at 65h..65h+65
            v_s = persist.tile([128, NKT, 4 * 65], BF16, tag="v")
            nc.vector.memset(
                v_s[:].rearrange("p k (h x) -> p k h x", x=65)[:, :, :, 64:65], 1.0)
            # normalized context, [q-part, 2 heads * 64 feat] per (qtile, pair)
            ctxn_s = [persist.tile([128, NKT, 128], BF16, tag=f"ctxn{i}", name=f"ctxn{i}")
                      for i in range(2)]
            # transposed normalized context [feat-part (2 heads), q] per pair
            ctxT_s = [persist.tile([128, S], BF16, tag=f"ctxT{i}", name=f"ctxT{i}")
                      for i in range(2)]
            # xv/xk stay fully resident (second pass over all 8 chunks)
            xv_s = persist.tile([128, 8, S], BF16, tag="xv")
            xk_s = persist.tile([128, 8, S], BF16, tag="xk")

            # ---- DMA order on the serialized DMA resource ----
            # sync queue: wq, bq, xq chunks, wk, tri/ident, xk chunks, wv, xv
            # scalar queue (emitted later, needed late): wo, bo4
            # wq chunk 0 first so the very first matmul can start ~1.2us sooner
            nc.sync.dma_start(wq_s[:, 0, :], wq[0:128, :])
            nc.sync.dma_start(bq_s[:], bq.ap().rearrange("(t p) o -> p (t o)", p=128))

            rs_in = [dram.tile([S // 2, D], F32, name=f"rs_in{i}") for i in range(2)]
            rs_out = [dram.tile([256, D], F32, name=f"rs_out{i}") for i in range(2)]

            # ================= phase A: projections + first strips =================
            # pre-warm the Exp table while the PE is still projecting
            warm = small.tile([128, 1], BF16, tag="warm")
            nc.scalar.activation(out=warm[:], in_=bq_s[:, 0:1].bitcast(BF16)[:, 0:1], func=Exp)

            # pr strips per (head, qgroup) — SBUF, persist across psum pools
            all_prs = {}

            def emit_strip(pool, h, qg, ki, chunked):
                """Scores + exp (+mask) for one (head, k-tile, q-group) strip."""
                pt, row = h // 2, 64 * (h % 2)
                q0, q1 = 1024 * qg, 1024 * (qg + 1)
                qstart = max(128 * ki, q0)
                w = q1 - qstart
                pr = probs.tile([128, 1024], BF16, tag="pr", name="pr")
                all_prs[(h, qg)][ki] = (pr, qstart)
                if chunked:  # [128, 512] tiles (1 bank, projection pool)
                    for c0 in range(0, w, 512):
                        cw = min(512, w - c0)
                        sck = pool.tile([128, 512], F32, tag="pp", name="sck")
                        nc.tensor.matmul(
                            sck[:, 0:cw],
                            kT_s[pt][row: row + 64, 128 * ki: 128 * (ki + 1)],
                            qT_s[pt][row: row + 64, qstart + c0: qstart + c0 + cw],
                            start=True, stop=True,
                        )
                        nc.scalar.activation(
                            out=pr[:, c0: c0 + cw], in_=sck[:, 0:cw], func=Exp,
                            scale=0.125,
                        )
                else:
                    sc = pool.tile([128, 1024], F32, tag="sc", name="sc")
                    for c0 in range(0, w, 512):
                        cw = min(512, w - c0)
                        nc.tensor.matmul(
                            sc[:, c0: c0 + cw],
                            kT_s[pt][row: row + 64, 128 * ki: 128 * (ki + 1)],
                            qT_s[pt][row: row + 64, qstart + c0: qstart + c0 + cw],
                            start=True, stop=True,
                        )
                    nc.scalar.activation(
                        out=pr[:, :w], in_=sc[:, :w], func=Exp, scale=0.125,
                    )
                if 128 * ki >= q0:  # diagonal block: causal mask
                    nc.vector.tensor_mul(pr[:, 0:128], pr[:, 0:128], tri_s[:])

            with tc.tile_pool(name="psProj", bufs=8, space="PSUM") as psP:
                # Q: both pt halves per chunk (transient xq)
                psq = {(pt, qb): psP.tile([128, 512], F32, tag="pp",
                                          name=f"psq{pt}{qb}")
                       for pt in range(2) for qb in range(4)}
                for kc in range(8):
                    xt = xin.tile([128, S], BF16, tag="x", name=f"xq{kc}")
                    nc.sync.dma_start(xt[:], xq[128 * kc: 128 * (kc + 1), :])
                    if kc < 7:  # next wq chunk rides between x chunks
                        nc.sync.dma_start(
                            wq_s[:, kc + 1, :], wq[128 * (kc + 1): 128 * (kc + 2), :])
                    for pt in range(2):
                        for qb in range(4):
                            nc.tensor.matmul(
                                psq[(pt, qb)][:],
                                wq_s[:, kc, 128 * pt: 128 * (pt + 1)],
                                xt[:, 512 * qb: 512 * (qb + 1)],
                                start=(kc == 0), stop=(kc == 7),
                            )
                nc.sync.dma_start(wk_s[:], wk.ap().rearrange("(kc p) f -> p kc f", p=128))
                nc.sync.dma_start(tri_s[:], tri.ap())
                nc.sync.dma_start(ident_s[:], ident.ap())
                for pt in range(2):
                    for qb in range(4):
                        nc.vector.tensor_scalar_add(
                            qT_s[pt][:, 512 * qb: 512 * (qb + 1)],
                            psq[(pt, qb)][:], bq_s[:, pt: pt + 1])

                # K-pt0 only: gates the first exp stream
                psk = {qb: psP.tile([128, 512], F32, tag="pp", name=f"psk0{qb}")
                       for qb in range(4)}
                for kc in range(8):
                    nc.sync.dma_start(xk_s[:, kc, :], xk[128 * kc: 128 * (kc + 1), :])
                    for qb in range(4):
                        nc.tensor.matmul(
                            psk[qb][:],
                            wk_s[:, kc, 0:128],
                            xk_s[:, kc, 512 * qb: 512 * (qb + 1)],
                            start=(kc == 0), stop=(kc == 7),
                        )
                # V input streams in right behind xk (needed from ~45us)
                nc.sync.dma_start(wv_s[:], wv.ap().rearrange("(kc p) f -> p kc f", p=128))
                for kc in range(8):
                    nc.sync.dma_start(xv_s[:, kc, :], xv[128 * kc: 128 * (kc + 1), :])
                for qb in range(4):  # split Act/DVE to shorten the critical path
                    eng = nc.scalar if qb < 2 else nc.vector
                    if eng is nc.scalar:
                        eng.copy(kT_s[0][:, 512 * qb: 512 * (qb + 1)], psk[qb][:])
                    else:
                        eng.tensor_copy(kT_s[0][:, 512 * qb: 512 * (qb + 1)], psk[qb][:])


                # first two heads' q-group-0 strips run inside the proj pool
                # (exp stream starts while K-pt1 projects), with K-pt1 chunk
                # bundles interleaved into the PE slack between strips
                all_prs[(0, 0)] = {}
                all_prs[(1, 0)] = {}
                for ki in range(8):
                    for h in (0, 1):
                        emit_strip(psP, h, 0, ki, chunked=True)
                    if ki % 2 == 1:  # K-pt1, one transient qb-bundle at a time
                        qb = ki // 2
                        kb1 = psP.tile([128, 512], F32, tag="pp", name=f"psk1{qb}")
                        for kc in range(8):
                            nc.tensor.matmul(
                                kb1[:],
                                wk_s[:, kc, 128:256],
                                xk_s[:, kc, 512 * qb: 512 * (qb + 1)],
                                start=(kc == 0), stop=(kc == 7),
                            )
                        nc.vector.tensor_copy(
                            kT_s[1][:, 512 * qb: 512 * (qb + 1)], kb1[:])

            # wo/bo arrive during attention (sync queue FIFO: after x traffic)
            nc.sync.dma_start(wo_s[:], wo.ap().rearrange("(c p) d -> p c d", p=128))
            nc.sync.dma_start(bo1_s[:], bo1.ap())
            nc.sync.dma_start(bo4_s[:], bo4.ap())

            # ================= phase B: attention + streamed oproj =================
            v_emitted = [0]

            def emit_v_bundle(psA):
                """Project V for one k-tile into natural layout (1 psum bank).
                An accumulation group must own its whole psum bank: start=True
                marks the full 2KB bank pending-zero."""
                ki = v_emitted[0]
                if ki >= NKT:
                    return
                v_emitted[0] += 1
                vp = psA.tile([128, 512], F32, tag="sa", name=f"vps{ki}")
                for kc in range(8):
                    nc.tensor.matmul(
                        vp[:, 0:256],
                        xv_s[:, kc, 128 * ki: 128 * (ki + 1)],
                        wv_s[:, kc, :],
                        start=(kc == 0), stop=(kc == 7),
                    )
                nc.vector.tensor_copy(
                    v_s[:, ki, :].rearrange("p (h x) -> p h x", x=65)[:, :, 0:64],
                    vp[:, 0:256].rearrange("p (h x) -> p h x", x=64),
                )

            with (
                tc.tile_pool(name="psS", bufs=2, space="PSUM") as psS,
                tc.tile_pool(name="psC", bufs=1, space="PSUM") as psC,
                tc.tile_pool(name="psA", bufs=2, space="PSUM") as psA,
                tc.tile_pool(name="psB", bufs=1, space="PSUM") as psB,
            ):
                def emit_rs(half):
                    if not os_mod.environ.get("BASS_SIM_NO_RS"):
                        nc.gpsimd.collective_compute(
                            "ReduceScatter", mybir.AluOpType.add,
                            replica_groups=[[0, 1, 2, 3], [4, 5, 6, 7]],
                            ins=[rs_in[half].opt()], outs=[rs_out[half].opt()],
                        )
                        nc.sync.dma_start(
                            out[256 * half: 256 * (half + 1), :], rs_out[half][:])
                    else:
                        nc.sync.dma_start(
                            out[256 * half: 256 * (half + 1), :],
                            rs_in[half][0:256, :])

                def emit_oproj(qi, eng_idx):
                    """po = ctxT0.T @ wo[0] + ctxT1.T @ wo[1] + bo4 for
                    q-tile qi, streamed during the attention exp window."""
                    ot = oout.tile([128, D], F32, tag="ot")
                    for dh in range(2):
                        po = psA.tile([128, 512], F32, tag="sa", name=f"po{qi}{dh}")
                        for p in range(2):
                            nc.tensor.matmul(
                                po[:],
                                ctxT_s[p][:, 128 * qi: 128 * (qi + 1)],
                                wo_s[:, p, 512 * dh: 512 * (dh + 1)],
                                start=(p == 0), stop=(p == 1),
                            )
                        nc.vector.tensor_add(
                            ot[:, 512 * dh: 512 * (dh + 1)], po[:],
                            bo4_s[:, 512 * dh: 512 * (dh + 1)])
                    half, sl = qi // 8, qi % 8
                    nc.sync.dma_start(
                        rs_in[half][128 * sl: 128 * (sl + 1), :], ot[:])
                    if sl == 7:
                        emit_rs(half)

                def emit_oproj_tail(qi):
                    """Tail q-tiles: po pairs in the (now free) scores pool,
                    bias via rank-1 matmul, drains split Act/DVE in parallel."""
                    pot = psS.tile([128, 1024], F32, tag="sc", name=f"pot{qi}")
                    for dh in range(2):
                        for p in range(2):
                            nc.tensor.matmul(
                                pot[:, 512 * dh: 512 * (dh + 1)],
                                ctxT_s[p][:, 128 * qi: 128 * (qi + 1)],
                                wo_s[:, p, 512 * dh: 512 * (dh + 1)],
                                start=(p == 0), stop=False,
                            )
                        nc.tensor.matmul(
                            pot[:, 512 * dh: 512 * (dh + 1)], ones1_s[:],
                            bo1_s[:, 512 * dh: 512 * (dh + 1)],
                            start=False, stop=True,
                        )
                    ot = oout.tile([128, D], F32, tag="ot")
                    if qi % 2 == 0:
                        nc.vector.tensor_copy(ot[:], pot[:])
                    else:
                        nc.scalar.copy(ot[:], pot[:])
                    nc.sync.dma_start(
                        rs_in[1][128 * (qi - 8): 128 * (qi - 7), :], ot[:])
                    if qi == 15:
                        emit_rs(1)

                def emit_avgroup(pair, qg, h, qi):
                    """ctx[q-part, 65] for q-tile qi: all strips k2 <= qi, then
                    normalize; on the pair's second head also transpose +
                    (pair 1) stream the output projection."""
                    heads = (2 * pair, 2 * pair + 1)
                    pt, row = h // 2, 64 * (h % 2)
                    prs = all_prs[(h, qg)]
                    if pair == 0 and h == heads[0]:
                        while v_emitted[0] <= qi:
                            emit_v_bundle(psA)
                    ctx = psC.tile([128, 65], F32, tag="ctx",
                                   name=f"ctx{pair}{qg}{h}{qi}")
                    for k2 in range(qi + 1):
                        pr2, qs2 = prs[k2]
                        off = 128 * qi - qs2
                        nc.tensor.matmul(
                            ctx[:],
                            pr2[:, off: off + 128],
                            v_s[:, k2, 65 * h: 65 * h + 65],
                            start=(k2 == 0), stop=(k2 == qi),
                        )
                    rc = small.tile([128, 1], F32, tag="rc")
                    nc.vector.reciprocal(rc[:], ctx[:, 64:65])
                    nc.vector.tensor_scalar_mul(
                        ctxn_s[pair][:, qi, row: row + 64], ctx[:, 0:64], rc[:])
                    if h == heads[1]:
                        tp = psB.tile([128, 128], BF16, tag="sb",
                                      name=f"tp{pair}{qi}")
                        nc.tensor.matmul(
                            tp[:], ctxn_s[pair][:, qi, :], ident_s[:],
                            is_transpose=True, start=True, stop=True,
                            skip_group_check=True,
                        )
                        nc.vector.tensor_copy(
                            ctxT_s[pair][:, 128 * qi: 128 * (qi + 1)], tp[:])
                        if pair == 1 and qi < 12:
                            emit_oproj(qi, qi)

                # (pair 0, qg 1): the deferred (pair 0, qg 0) AV bursts are
                # spread across the first 8 strips (whose exps they don't
                # depend on — keeps the pr pool rotation acyclic), V bundles
                # ride along; inline AV groups take the last 8 strips
                for h in (0, 1):
                    all_prs[(h, 1)] = {}
                    for ki in range(8):
                        emit_strip(psS, h, 1, ki, chunked=False)
                        if h == 0:
                            emit_v_bundle(psA)
                        emit_avgroup(0, 0, h, ki)
                    for ki in range(8, 16):
                        emit_strip(psS, h, 1, ki, chunked=False)
                        emit_avgroup(0, 1, h, ki)

                for pair in range(2):
                    heads = (2 * pair, 2 * pair + 1)
                    for qg in range(2):
                        for h in heads:
                            if pair == 0:
                                continue  # already emitted above
                            all_prs[(h, qg)] = {}
                            for ki in range(8 * qg + 8):
                                emit_strip(psS, h, qg, ki, chunked=False)
                                if ki >= 8 * qg:
                                    emit_avgroup(pair, qg, h, ki)
                                    # deferred tail oproj, one step behind the
                                    # avgroup that completes its ctxT
                                    if qg == 1 and h == heads[1] and ki >= 13:
                                        emit_oproj_tail(ki - 1)

                emit_oproj_tail(15)


            if dbg:
                for i in range(2):
                    nc.sync.dma_start(dbg_qT[:, i, :], qT_s[i][:])
                    nc.sync.dma_start(dbg_kT[:, i, :], kT_s[i][:])
                    nc.sync.dma_start(dbg_ctxn[:, i, :, :], ctxn_s[i][:])
                    nc.sync.dma_start(dbg_ctxT[:, i, :], ctxT_s[i][:])
                nc.sync.dma_start(dbg_v[:], v_s[:])

    nc.compile()
    return nc


def _prep_inputs(query, key_, value, w_q, b_q, w_k, b_k, w_v, b_v, w_o, b_o):
    """Build the 8 per-core input maps (host-side sharding / re-layout)."""
    import ml_dtypes
    f32 = np.float32
    bf16 = ml_dtypes.bfloat16

    r = np.arange(128)[:, None]
    j = np.arange(128)[None, :]
    tri = (j >= r).astype(bf16)  # allowed (q >= k) within diagonal block
    ident = np.eye(128, dtype=bf16)

    wqT = np.ascontiguousarray(np.asarray(w_q, f32).T)  # [D_in, D_out]
    wkT = np.ascontiguousarray(np.asarray(w_k, f32).T)
    wvT = np.ascontiguousarray(np.asarray(w_v, f32).T)
    woT = np.ascontiguousarray(np.asarray(w_o, f32).T)  # [D_in(=feat), D_out]
    b_q = np.asarray(b_q, f32)
    b_v = np.asarray(b_v, f32)
    b_o = np.asarray(b_o, f32)

    xT = {}
    for g in range(B):
        xT[("q", g)] = np.ascontiguousarray(np.asarray(query[g], f32).T.astype(bf16))
        xT[("k", g)] = np.ascontiguousarray(np.asarray(key_[g], f32).T.astype(bf16))
        xT[("v", g)] = np.ascontiguousarray(np.asarray(value[g], f32).T.astype(bf16))

    in_maps = []
    for c in range(N_CORES):
        g, p = c // 4, c % 4
        fsel = slice(FPC * p, FPC * (p + 1))
        # b_v folded into the output bias: ctx_norm contains +b_v, so
        # out partial += b_v[fsel] @ w_o.T[fsel, :]; b_o/4 spread over 4 cores.
        bo_eff = b_o / 4.0 + b_v[fsel] @ woT[fsel, :]
        bo4 = np.broadcast_to(bo_eff.astype(f32), (128, D)).copy()
        in_maps.append({
            "xq": xT[("q", g)],
            "xk": xT[("k", g)],
            "xv": xT[("v", g)],
            "wq": np.ascontiguousarray(wqT[:, fsel].astype(bf16)),
            "wk": np.ascontiguousarray(wkT[:, fsel].astype(bf16)),
            "wv": np.ascontiguousarray(wvT[:, fsel].astype(bf16)),
            "wo": np.ascontiguousarray(woT[fsel, :].astype(bf16)),
            "bq": np.ascontiguousarray(b_q[fsel].reshape(FPC, 1)),
            "bo1": np.ascontiguousarray(bo_eff.reshape(1, D).astype(bf16)),
            "bo4": bo4,
            "tri": tri,
            "ident": ident,
        })
    return in_maps


def run(inputs, trace=False):
    from concourse.bass_utils import run_bass_kernel_spmd

    if "nc" not in _CACHE:
        _CACHE["nc"] = _build_nc()
    nc = _CACHE["nc"]
    in_maps = _prep_inputs(
        inputs["query"], inputs["key_"], inputs["value"],
        inputs["w_q"], inputs["b_q"], inputs["w_k"], inputs["b_k"],
        inputs["w_v"], inputs["b_v"], inputs["w_o"], inputs["b_o"],
    )
    res = run_bass_kernel_spmd(
        nc, in_maps, core_ids=list(range(N_CORES)), trace=trace,
    )
    out = np.empty((B, S, D), np.float32)
    for c in range(N_CORES):
        g, p = c // 4, c % 4
        # RS half i scatters q rows [1024*i + 256*p, 1024*i + 256*(p+1))
        out[g, 256 * p: 256 * (p + 1), :] = res.results[c]["out"][0:256]
        out[g, 1024 + 256 * p: 1024 + 256 * (p + 1), :] = res.results[c]["out"][256:512]
    return out, res


def kernel(**inputs):
    out, _ = run(inputs, trace=False)
    return out


# revision 57
# speedup vs baseline: 1.0057x; 1.0057x over previous
"""Causal multi-head attention on 8 Trainium2 NeuronCores.

Sharding: core c -> (batch g = c // 4, head-group p = c % 4, heads 4p..4p+3).
Each core projects Q/K/V for its batch with its 256 feature columns
(column-sharded w_q/w_k/w_v), runs causal attention for its 4 heads, computes
the partial output projection with its 256 rows of w_o, and a ReduceScatter
over each batch group sums the partials and hands every core its own 512-row
output shard.

v2 design (everything bf16 into the PE, fp32 accumulation):
- Exact 128-granularity causality: scores strips [128 kpos, q>=128*ki] per
  (head, k-tile), 53% of the S^2 matrix instead of 62.5%.
- Flipped AV: ctx[q-part, 65] += pr_block.T @ v_aug where v_aug is V in
  natural [kpos, feat] layout with an appended ones column -> softmax
  denominator lands in psum column 64, per-PARTITION normalize (cheap).
- V projected directly in natural layout (no PE transpose pass).
- b_k dropped entirely (softmax-shift invariant), b_v folded into the output
  bias host-side (bv @ w_o), b_q applied as the exp() per-partition bias via
  k.bq matvecs.
- Streamed tail: per q-tile normalize -> PE transpose -> output projection
  -> rs_in DMA, overlapped under the attention exp stream.
"""

import numpy as np

B, S, D, H = 2, 2048, 1024, 16
DK = D // H  # 64
N_CORES = 8
FPC = 256  # features per core
NKT = S // 128  # 16 k/q tiles

_CACHE = {}


def _build_nc():
    import os as os_mod
    import concourse.mybir as mybir
    import concourse.tile as tile
    from concourse import bacc

    F32 = mybir.dt.float32
    BF16 = mybir.dt.bfloat16
    Exp = mybir.ActivationFunctionType.Exp

    nc = bacc.Bacc("TRN2", target_bir_lowering=False, debug=False, num_devices=8)

    xq = nc.dram_tensor("xq", [D, S], BF16, kind="ExternalInput")
    xk = nc.dram_tensor("xk", [D, S], BF16, kind="ExternalInput")
    xv = nc.dram_tensor("xv", [D, S], BF16, kind="ExternalInput")
    wq = nc.dram_tensor("wq", [D, FPC], BF16, kind="ExternalInput")
    wk = nc.dram_tensor("wk", [D, FPC], BF16, kind="ExternalInput")
    wv = nc.dram_tensor("wv", [D, FPC], BF16, kind="ExternalInput")
    wo = nc.dram_tensor("wo", [FPC, D], BF16, kind="ExternalInput")
    bq = nc.dram_tensor("bq", [FPC, 1], F32, kind="ExternalInput")
    bo1 = nc.dram_tensor("bo1", [1, D], BF16, kind="ExternalInput")
    bo4 = nc.dram_tensor("bo4", [128, D], F32, kind="ExternalInput")
    tri = nc.dram_tensor("tri", [128, 128], BF16, kind="ExternalInput")
    ident = nc.dram_tensor("ident", [128, 128], BF16, kind="ExternalInput")
    out = nc.dram_tensor("out", [512, D], F32, kind="ExternalOutput")

    dbg = bool(os_mod.environ.get("BASS_DEBUG_DUMP"))
    if dbg:
        dbg_qT = nc.dram_tensor("dbg_qT", [128, 2, S], BF16, kind="ExternalOutput")
        dbg_kT = nc.dram_tensor("dbg_kT", [128, 2, S], BF16, kind="ExternalOutput")
        dbg_v = nc.dram_tensor("dbg_v", [128, NKT, 4 * 65], BF16, kind="ExternalOutput")
        dbg_ctxn = nc.dram_tensor("dbg_ctxn", [128, 2, NKT, 128], BF16, kind="ExternalOutput")
        dbg_ctxT = nc.dram_tensor("dbg_ctxT", [128, 2, S], BF16, kind="ExternalOutput")

    from contextlib import ExitStack
    with tile.TileContext(nc) as tc:
        with (
            tc.tile_pool(name="consts", bufs=1) as consts,
            tc.tile_pool(name="persist", bufs=1) as persist,
            tc.tile_pool(name="xin", bufs=3) as xin,
            tc.tile_pool(name="probs", bufs=26) as probs,
            tc.tile_pool(name="small", bufs=6) as small,
            tc.tile_pool(name="oout", bufs=4) as oout,
            tc.tile_pool(name="dram", bufs=1, space="DRAM") as dram,
        ):
            # ---- constant tiles ----
            wq_s = consts.tile([128, 8, FPC], BF16, tag="wq")
            wk_s = consts.tile([128, 8, FPC], BF16, tag="wk")
            wv_s = consts.tile([128, 8, FPC], BF16, tag="wv")
            wo_s = consts.tile([128, 2, D], BF16, tag="wo")
            bq_s = consts.tile([128, 2], F32, tag="bq")
            bo1_s = consts.tile([1, D], BF16, tag="bo1")
            bo4_s = consts.tile([128, D], F32, tag="bo4")
            ones1_s = consts.tile([1, 128], BF16, tag="ones1")
            tri_s = consts.tile([128, 128], BF16, tag="tri")
            ident_s = consts.tile([128, 128], BF16, tag="ident")
            nc.vector.memset(ones1_s[:], 1.0)


            # ---- persistent activations ----
            # qT/kT: [head-dim on partitions (2 heads: rows 0:64 / 64:128), S]
            qT_s = [persist.tile([128, S], BF16, tag=f"qT{i}", name=f"qT{i}") for i in range(2)]
            kT_s = [persist.tile([128, S], BF16, tag=f"kT{i}", name=f"kT{i}") for i in range(2)]
            # V natural layout + ones column: per ki, head h strip at 65h..65h+65
            v_s = persist.tile([128, NKT, 4 * 65], BF16, tag="v")
            nc.vector.memset(
                v_s[:].rearrange("p k (h x) -> p k h x", x=65)[:, :, :, 64:65], 1.0)
            # normalized context, [q-part, 2 heads * 64 feat] per (qtile, pair)
            ctxn_s = [persist.tile([128, NKT, 128], BF16, tag=f"ctxn{i}", name=f"ctxn{i}")
                      for i in range(2)]
            # transposed normalized context [feat-part (2 heads), q] per pair
            ctxT_s = [persist.tile([128, S], BF16, tag=f"ctxT{i}", name=f"ctxT{i}")
                      for i in range(2)]
            # xv/xk stay fully resident (second pass over all 8 chunks)
            xv_s = persist.tile([128, 8, S], BF16, tag="xv")
            xk_s = persist.tile([128, 8, S], BF16, tag="xk")

            # ---- DMA order on the serialized DMA resource ----
            # sync queue: wq, bq, xq chunks, wk, tri/ident, xk chunks, wv, xv
            # scalar queue (emitted later, needed late): wo, bo4
            # wq chunk 0 first so the very first matmul can start ~1.2us sooner
            nc.sync.dma_start(wq_s[:, 0, :], wq[0:128, :])
            nc.sync.dma_start(bq_s[:], bq.ap().rearrange("(t p) o -> p (t o)", p=128))

            rs_in = [dram.tile([S // 2, D], F32, name=f"rs_in{i}") for i in range(2)]
            rs_out = [dram.tile([256, D], F32, name=f"rs_out{i}") for i in range(2)]

            # ================= phase A: projections + first strips =================
            # pre-warm the Exp table while the PE is still projecting
            warm = small.tile([128, 1], BF16, tag="warm")
            nc.scalar.activation(out=warm[:], in_=bq_s[:, 0:1].bitcast(BF16)[:, 0:1], func=Exp)

            # pr strips per (head, qgroup) — SBUF, persist across psum pools
            all_prs = {}

            def emit_strip(pool, h, qg, ki, chunked):
                """Scores + exp (+mask) for one (head, k-tile, q-group) strip."""
                pt, row = h // 2, 64 * (h % 2)
                q0, q1 = 1024 * qg, 1024 * (qg + 1)
                qstart = max(128 * ki, q0)
                w = q1 - qstart
                pr = probs.tile([128, 1024], BF16, tag="pr", name="pr")
                all_prs[(h, qg)][ki] = (pr, qstart)
                if chunked:  # [128, 512] tiles (1 bank, projection pool)
                    for c0 in range(0, w, 512):
                        cw = min(512, w - c0)
                        sck = pool.tile([128, 512], F32, tag="pp", name="sck")
                        nc.tensor.matmul(
                            sck[:, 0:cw],
                            kT_s[pt][row: row + 64, 128 * ki: 128 * (ki + 1)],
                            qT_s[pt][row: row + 64, qstart + c0: qstart + c0 + cw],
                            start=True, stop=True,
                        )
                        nc.scalar.activation(
                            out=pr[:, c0: c0 + cw], in_=sck[:, 0:cw], func=Exp,
                            scale=0.125,
                        )
                else:
                    sc = pool.tile([128, 1024], F32, tag="sc", name="sc")
                    for c0 in range(0, w, 512):
                        cw = min(512, w - c0)
                        nc.tensor.matmul(
                            sc[:, c0: c0 + cw],
                            kT_s[pt][row: row + 64, 128 * ki: 128 * (ki + 1)],
                            qT_s[pt][row: row + 64, qstart + c0: qstart + c0 + cw],
                            start=True, stop=True,
                        )
                    nc.scalar.activation(
                        out=pr[:, :w], in_=sc[:, :w], func=Exp, scale=0.125,
                    )
                if 128 * ki >= q0:  # diagonal block: causal mask
                    nc.vector.tensor_mul(pr[:, 0:128], pr[:, 0:128], tri_s[:])

            with tc.tile_pool(name="psProj", bufs=8, space="PSUM") as psP:
                # Q: both pt halves per chunk (transient xq)
                psq = {(pt, qb): psP.tile([128, 512], F32, tag="pp",
                                          name=f"psq{pt}{qb}")
                       for pt in range(2) for qb in range(4)}
                for kc in range(8):
                    xt = xin.tile([128, S], BF16, tag="x", name=f"xq{kc}")
                    nc.sync.dma_start(xt[:], xq[128 * kc: 128 * (kc + 1), :])
                    if kc < 7:  # next wq chunk rides between x chunks
                        nc.sync.dma_start(
                            wq_s[:, kc + 1, :], wq[128 * (kc + 1): 128 * (kc + 2), :])
                    for pt in range(2):
                        for qb in range(4):
                            nc.tensor.matmul(
                                psq[(pt, qb)][:],
                                wq_s[:, kc, 128 * pt: 128 * (pt + 1)],
                                xt[:, 512 * qb: 512 * (qb + 1)],
                                start=(kc == 0), stop=(kc == 7),
                            )
                nc.sync.dma_start(wk_s[:], wk.ap().rearrange("(kc p) f -> p kc f", p=128))
                nc.sync.dma_start(tri_s[:], tri.ap())
                nc.sync.dma_start(ident_s[:], ident.ap())
                for pt in range(2):
                    for qb in range(4):
                        nc.vector.tensor_scalar_add(
                            qT_s[pt][:, 512 * qb: 512 * (qb + 1)],
                            psq[(pt, qb)][:], bq_s[:, pt: pt + 1])

                # K-pt0 only: gates the first exp stream
                psk = {qb: psP.tile([128, 512], F32, tag="pp", name=f"psk0{qb}")
                       for qb in range(4)}
                for kc in range(8):
                    nc.sync.dma_start(xk_s[:, kc, :], xk[128 * kc: 128 * (kc + 1), :])
                    for qb in range(4):
                        nc.tensor.matmul(
                            psk[qb][:],
                            wk_s[:, kc, 0:128],
                            xk_s[:, kc, 512 * qb: 512 * (qb + 1)],
                            start=(kc == 0), stop=(kc == 7),
                        )
                # V input streams in right behind xk (needed from ~45us)
                nc.sync.dma_start(wv_s[:], wv.ap().rearrange("(kc p) f -> p kc f", p=128))
                for kc in range(8):
                    nc.sync.dma_start(xv_s[:, kc, :], xv[128 * kc: 128 * (kc + 1), :])
                for qb in range(4):  # split Act/DVE to shorten the critical path
                    eng = nc.scalar if qb < 2 else nc.vector
                    if eng is nc.scalar:
                        eng.copy(kT_s[0][:, 512 * qb: 512 * (qb + 1)], psk[qb][:])
                    else:
                        eng.tensor_copy(kT_s[0][:, 512 * qb: 512 * (qb + 1)], psk[qb][:])


                # first two heads' q-group-0 strips run inside the proj pool
                # (exp stream starts while K-pt1 projects), with K-pt1 chunk
                # bundles interleaved into the PE slack between strips
                all_prs[(0, 0)] = {}
                all_prs[(1, 0)] = {}
                for ki in range(8):
                    for h in (0, 1):
                        emit_strip(psP, h, 0, ki, chunked=True)
                    if ki % 2 == 1:  # K-pt1, one transient qb-bundle at a time
                        qb = ki // 2
                        kb1 = psP.tile([128, 512], F32, tag="pp", name=f"psk1{qb}")
                        for kc in range(8):
                            nc.tensor.matmul(
                                kb1[:],
                                wk_s[:, kc, 128:256],
                                xk_s[:, kc, 512 * qb: 512 * (qb + 1)],
                                start=(kc == 0), stop=(kc == 7),
                            )
                        nc.vector.tensor_copy(
                            kT_s[1][:, 512 * qb: 512 * (qb + 1)], kb1[:])

            # wo/bo arrive during attention (sync queue FIFO: after x traffic)
            nc.sync.dma_start(wo_s[:], wo.ap().rearrange("(c p) d -> p c d", p=128))
            nc.sync.dma_start(bo1_s[:], bo1.ap())
            nc.sync.dma_start(bo4_s[:], bo4.ap())

            # ================= phase B: attention + streamed oproj =================
            v_emitted = [0]

            def emit_v_bundle(psA):
                """Project V for one k-tile into natural layout (1 psum bank).
                An accumulation group must own its whole psum bank: start=True
                marks the full 2KB bank pending-zero."""
                ki = v_emitted[0]
                if ki >= NKT:
                    return
                v_emitted[0] += 1
                vp = psA.tile([128, 512], F32, tag="sa", name=f"vps{ki}")
                for kc in range(8):
                    nc.tensor.matmul(
                        vp[:, 0:256],
                        xv_s[:, kc, 128 * ki: 128 * (ki + 1)],
                        wv_s[:, kc, :],
                        start=(kc == 0), stop=(kc == 7),
                    )
                nc.vector.tensor_copy(
                    v_s[:, ki, :].rearrange("p (h x) -> p h x", x=65)[:, :, 0:64],
                    vp[:, 0:256].rearrange("p (h x) -> p h x", x=64),
                )

            with (
                tc.tile_pool(name="psS", bufs=2, space="PSUM") as psS,
                tc.tile_pool(name="psC", bufs=1, space="PSUM") as psC,
                tc.tile_pool(name="psA", bufs=2, space="PSUM") as psA,
                tc.tile_pool(name="psB", bufs=1, space="PSUM") as psB,
            ):
                def emit_rs(half):
                    if not os_mod.environ.get("BASS_SIM_NO_RS"):
                        nc.gpsimd.collective_compute(
                            "ReduceScatter", mybir.AluOpType.add,
                            replica_groups=[[0, 1, 2, 3], [4, 5, 6, 7]],
                            ins=[rs_in[half].opt()], outs=[rs_out[half].opt()],
                        )
                        nc.sync.dma_start(
                            out[256 * half: 256 * (half + 1), :], rs_out[half][:])
                    else:
                        nc.sync.dma_start(
                            out[256 * half: 256 * (half + 1), :],
                            rs_in[half][0:256, :])

                def emit_oproj(qi, eng_idx):
                    """po = ctxT0.T @ wo[0] + ctxT1.T @ wo[1] + bo4 for
                    q-tile qi, streamed during the attention exp window."""
                    ot = oout.tile([128, D], F32, tag="ot")
                    for dh in range(2):
                        po = psA.tile([128, 512], F32, tag="sa", name=f"po{qi}{dh}")
                        for p in range(2):
                            nc.tensor.matmul(
                                po[:],
                                ctxT_s[p][:, 128 * qi: 128 * (qi + 1)],
                                wo_s[:, p, 512 * dh: 512 * (dh + 1)],
                                start=(p == 0), stop=(p == 1),
                            )
                        nc.vector.tensor_add(
                            ot[:, 512 * dh: 512 * (dh + 1)], po[:],
                            bo4_s[:, 512 * dh: 512 * (dh + 1)])
                    half, sl = qi // 8, qi % 8
                    nc.sync.dma_start(
                        rs_in[half][128 * sl: 128 * (sl + 1), :], ot[:])
                    if sl == 7:
                        emit_rs(half)

                def emit_oproj_tail(qi):
                    """Tail q-tiles: po pairs in the (now free) scores pool,
                    bias via rank-1 matmul, drains split Act/DVE in parallel."""
                    pot = psS.tile([128, 1024], F32, tag="sc", name=f"pot{qi}")
                    for dh in range(2):
                        for p in range(2):
                            nc.tensor.matmul(
                                pot[:, 512 * dh: 512 * (dh + 1)],
                                ctxT_s[p][:, 128 * qi: 128 * (qi + 1)],
                                wo_s[:, p, 512 * dh: 512 * (dh + 1)],
                                start=(p == 0), stop=False,
                            )
                        nc.tensor.matmul(
                            pot[:, 512 * dh: 512 * (dh + 1)], ones1_s[:],
                            bo1_s[:, 512 * dh: 512 * (dh + 1)],
                            start=False, stop=True,
                        )
                    ot = oout.tile([128, D], F32, tag="ot")
                    nc.scalar.copy(ot[:], pot[:])
                    nc.sync.dma_start(
                        rs_in[1][128 * (qi - 8): 128 * (qi - 7), :], ot[:])
                    if qi == 15:
                        emit_rs(1)

                def emit_avgroup(pair, qg, h, qi):
                    """ctx[q-part, 65] for q-tile qi: all strips k2 <= qi, then
                    normalize; on the pair's second head also transpose +
                    (pair 1) stream the output projection."""
                    heads = (2 * pair, 2 * pair + 1)
                    pt, row = h // 2, 64 * (h % 2)
                    prs = all_prs[(h, qg)]
                    if pair == 0 and h == heads[0]:
                        while v_emitted[0] <= qi:
                            emit_v_bundle(psA)
                    ctx = psC.tile([128, 65], F32, tag="ctx",
                                   name=f"ctx{pair}{qg}{h}{qi}")
                    for k2 in range(qi + 1):
                        pr2, qs2 = prs[k2]
                        off = 128 * qi - qs2
                        nc.tensor.matmul(
                            ctx[:],
                            pr2[:, off: off + 128],
                            v_s[:, k2, 65 * h: 65 * h + 65],
                            start=(k2 == 0), stop=(k2 == qi),
                        )
                    rc = small.tile([128, 1], F32, tag="rc")
                    nc.vector.reciprocal(rc[:], ctx[:, 64:65])
                    nc.vector.tensor_scalar_mul(
                        ctxn_s[pair][:, qi, row: row + 64], ctx[:, 0:64], rc[:])
                    if h == heads[1]:
                        tp = psB.tile([128, 128], BF16, tag="sb",
                                      name=f"tp{pair}{qi}")
                        nc.tensor.matmul(
                            tp[:], ctxn_s[pair][:, qi, :], ident_s[:],
                            is_transpose=True, start=True, stop=True,
                            skip_group_check=True,
                        )
                        nc.vector.tensor_copy(
                            ctxT_s[pair][:, 128 * qi: 128 * (qi + 1)], tp[:])
                        if pair == 1 and qi < 12:
                            emit_oproj(qi, qi)

                # (pair 0, qg 1): the deferred (pair 0, qg 0) AV bursts are
                # spread across the first 8 strips (whose exps they don't
                # depend on — keeps the pr pool rotation acyclic), V bundles
                # ride along; inline AV groups take the last 8 strips
                for h in (0, 1):
                    all_prs[(h, 1)] = {}
                    for ki in range(8):
                        emit_strip(psS, h, 1, ki, chunked=False)
                        if h == 0:
                            emit_v_bundle(psA)
                        emit_avgroup(0, 0, h, ki)
                    for ki in range(8, 16):
                        emit_strip(psS, h, 1, ki, chunked=False)
                        emit_avgroup(0, 1, h, ki)

                for pair in range(2):
                    heads = (2 * pair, 2 * pair + 1)
                    for qg in range(2):
                        for h in heads:
                            if pair == 0:
                                continue  # already emitted above
                            all_prs[(h, qg)] = {}
                            for ki in range(8 * qg + 8):
                                emit_strip(psS, h, qg, ki, chunked=False)
                                if ki >= 8 * qg:
                                    emit_avgroup(pair, qg, h, ki)
                                    # deferred tail oproj, one step behind the
                                    # avgroup that completes its ctxT
                                    if qg == 1 and h == heads[1] and ki >= 13:
                                        emit_oproj_tail(ki - 1)

                emit_oproj_tail(15)


            if dbg:
                for i in range(2):
                    nc.sync.dma_start(dbg_qT[:, i, :], qT_s[i][:])
                    nc.sync.dma_start(dbg_kT[:, i, :], kT_s[i][:])
                    nc.sync.dma_start(dbg_ctxn[:, i, :, :], ctxn_s[i][:])
                    nc.sync.dma_start(dbg_ctxT[:, i, :], ctxT_s[i][:])
                nc.sync.dma_start(dbg_v[:], v_s[:])

    nc.compile()
    return nc


def _prep_inputs(query, key_, value, w_q, b_q, w_k, b_k, w_v, b_v, w_o, b_o):
    """Build the 8 per-core input maps (host-side sharding / re-layout)."""
    import ml_dtypes
    f32 = np.float32
    bf16 = ml_dtypes.bfloat16

    r = np.arange(128)[:, None]
    j = np.arange(128)[None, :]
    tri = (j >= r).astype(bf16)  # allowed (q >= k) within diagonal block
    ident = np.eye(128, dtype=bf16)

    wqT = np.ascontiguousarray(np.asarray(w_q, f32).T)  # [D_in, D_out]
    wkT = np.ascontiguousarray(np.asarray(w_k, f32).T)
    wvT = np.ascontiguousarray(np.asarray(w_v, f32).T)
    woT = np.ascontiguousarray(np.asarray(w_o, f32).T)  # [D_in(=feat), D_out]
    b_q = np.asarray(b_q, f32)
    b_v = np.asarray(b_v, f32)
    b_o = np.asarray(b_o, f32)

    xT = {}
    for g in range(B):
        xT[("q", g)] = np.ascontiguousarray(np.asarray(query[g], f32).T.astype(bf16))
        xT[("k", g)] = np.ascontiguousarray(np.asarray(key_[g], f32).T.astype(bf16))
        xT[("v", g)] = np.ascontiguousarray(np.asarray(value[g], f32).T.astype(bf16))

    in_maps = []
    for c in range(N_CORES):
        g, p = c // 4, c % 4
        fsel = slice(FPC * p, FPC * (p + 1))
        # b_v folded into the output bias: ctx_norm contains +b_v, so
        # out partial += b_v[fsel] @ w_o.T[fsel, :]; b_o/4 spread over 4 cores.
        bo_eff = b_o / 4.0 + b_v[fsel] @ woT[fsel, :]
        bo4 = np.broadcast_to(bo_eff.astype(f32), (128, D)).copy()
        in_maps.append({
            "xq": xT[("q", g)],
            "xk": xT[("k", g)],
            "xv": xT[("v", g)],
            "wq": np.ascontiguousarray(wqT[:, fsel].astype(bf16)),
            "wk": np.ascontiguousarray(wkT[:, fsel].astype(bf16)),
            "wv": np.ascontiguousarray(wvT[:, fsel].astype(bf16)),
            "wo": np.ascontiguousarray(woT[fsel, :].astype(bf16)),
            "bq": np.ascontiguousarray(b_q[fsel].reshape(FPC, 1)),
            "bo1": np.ascontiguousarray(bo_eff.reshape(1, D).astype(bf16)),
            "bo4": bo4,
            "tri": tri,
            "ident": ident,
        })
    return in_maps


def run(inputs, trace=False):
    from concourse.bass_utils import run_bass_kernel_spmd

    if "nc" not in _CACHE:
        _CACHE["nc"] = _build_nc()
    nc = _CACHE["nc"]
    in_maps = _prep_inputs(
        inputs["query"], inputs["key_"], inputs["value"],
        inputs["w_q"], inputs["b_q"], inputs["w_k"], inputs["b_k"],
        inputs["w_v"], inputs["b_v"], inputs["w_o"], inputs["b_o"],
    )
    res = run_bass_kernel_spmd(
        nc, in_maps, core_ids=list(range(N_CORES)), trace=trace,
    )
    out = np.empty((B, S, D), np.float32)
    for c in range(N_CORES):
        g, p = c // 4, c % 4
        # RS half i scatters q rows [1024*i + 256*p, 1024*i + 256*(p+1))
        out[g, 256 * p: 256 * (p + 1), :] = res.results[c]["out"][0:256]
        out[g, 1024 + 256 * p: 1024 + 256 * (p + 1), :] = res.results[c]["out"][256:512]
    return out, res


def kernel(**inputs):
    out, _ = run(inputs, trace=False)
    return out
